# revision 1
# baseline (speedup 1.0000x reference)
"""Trainium2 Bass kernel for nn_Autoregression (16-state AR whitening log-prob).

Math: reference computes log_prob[b,k,t] = -0.5*(C*log(2pi) + logdet(Sigma_k)
+ es_k(t)^T Sigma_k^{-1} es_k(t)) with es = causal_conv(x, W, b).  Since
Sigma^{-1} = L^{-T} L^{-1} and es is affine in x, fold L^{-1} into the conv:
W2 = L^{-1} W, b2 = L^{-1} b, then mahalanobis = sum_c conv(x; W2, b2)^2.

Device layout (per core, T sharded 8 ways with an 8-sample left halo):
conv as matmuls over 128-t chunks producing PSUM [128 t, 512 (8 states x 64
ch)] x 2 halves; contraction packed as (c_in x 2 time-shifts)=128 rows per
step, 4 steps + a 65-row step for the j=8 tap whose ones-row carries the
bias.  ACT squares PSUM -> bf16 SBUF; DVE does the per-state segmented
reduce [128, 8, 64] -> [128, 8]; a small PE transpose flips [128 t, 16 k]
-> [16 k, 128 t]; DVE applies -0.5 and the per-state constant; DMA out.
"""

import os

import numpy as np
import ml_dtypes

import concourse.bass as bass
import concourse.bacc as bacc_mod
import concourse.mybir as mybir
import concourse.tile as tile
from concourse.bass_utils import run_bass_kernel_spmd
from concourse.tile_rust import add_dep_helper

K = 16          # states
C = 64          # channels
T = 65536       # time
AR = 8          # ar order (kernel size AR+1)
NCORES = 8
TLOC = T // NCORES          # 8192 outputs per core
TC = 128                    # outputs per chunk (matmul M)
WAVE = 16                   # chunks per wave (input tile granularity)
WCOLS = TC * WAVE           # 2048 outputs per wave
NW = TLOC // WCOLS          # waves per core
KP = K // 2
NSTEP = 5                   # contraction steps: 4 full + 1 (j=8 + bias row)
NH = 2                      # psum halves (states 0-7, 8-15)

MM_DT = mybir.dt.bfloat16   # conv matmul dtype
SQ_DT = mybir.dt.bfloat16   # squares dtype

_MM_NP = mybir.dt.np(MM_DT)

_CACHE: dict = {}


def _build_program():
    nc = bacc_mod.Bacc()
    f32 = mybir.dt.float32

    # xin rows 0-63: x slice (with halo); rows 64-127: same shifted left by 1
    # (host-duplicated so each wave's xd tile loads with a single DMA)
    xin = nc.declare_dram_parameter("xin", [128, TLOC + AR], MM_DT, isOutput=False)
    # weights as the matmul moving operand: [contraction, step, (half, state, ch)]
    wts = nc.declare_dram_parameter("wts", [128, NSTEP, 1024], MM_DT, isOutput=False)
    ident = nc.declare_dram_parameter("ident", [128, 128], mybir.dt.float32r, isOutput=False)
    biasc = nc.declare_dram_parameter("biasc", [K, 1], f32, isOutput=False)
    onesd = nc.declare_dram_parameter("onesd", [1, WCOLS], MM_DT, isOutput=False)
    out = nc.declare_dram_parameter("out", [K, TLOC], f32, isOutput=True)

    XDW = WCOLS + AR

    with tile.TileContext(nc) as tc:
        with (
            tc.tile_pool(name="singles", bufs=1) as singles,
            # one slot per wave: input DMAs never wait (no slot WAR/WAW)
            tc.tile_pool(name="xpool", bufs=NW) as xpool,
            tc.tile_pool(name="sqpool", bufs=12) as sqpool,
            tc.tile_pool(name="mpool", bufs=6) as mpool,
            tc.tile_pool(name="conv_ps", bufs=5, space="PSUM") as conv_ps,
            tc.tile_pool(name="mt_ps", bufs=2, space="PSUM") as mt_ps,
            tc.tile_pool(name="obs_ps", bufs=1, space="PSUM") as obs_ps,
        ):
            # Matmuls must never be the first PE instruction to observe more
            # than one producer semaphore (1-wait ISA slots; bacc's event-sem
            # legalization costs sequencer time).  pe_observe() emits a tiny
            # 2x2 "reader" matmul whose operands come from a single
            # producer's tile; ordering edges pin readers ahead of the next
            # real matmul.
            scratch = obs_ps.tile([2, 128], f32)
            scratch2 = singles.tile([2, 128], SQ_DT)
            nc.vector.memset(scratch2, 0.0)
            pending = []
            obs_after = [None]

            def pe_observe(col):
                i = nc.tensor.matmul(
                    scratch[0:2, 0:2], col, col, start=True, stop=True
                )
                if obs_after[0] is not None:
                    # not earlier than late in the previous wave, or the PE
                    # FIFO head-of-line blocks on a DMA that hasn't landed
                    add_dep_helper(i.ins, obs_after[0].ins, sync=False)
                pending.append(i)

            def _flush(i):
                while pending:
                    add_dep_helper(i.ins, pending.pop().ins, sync=False)
                return i

            def pe_matmul(*args, **kw):
                return _flush(nc.tensor.matmul(*args, **kw))

            # dep-free warmup matmuls: keep the PE busy through the initial
            # input DMAs so HAM un-throttles before real work (N=128 streams
            # so the activity monitor sees a busy array)
            for _ in range(35):
                nc.tensor.matmul(
                    scratch[0:2, 0:128],
                    scratch2[0:2, 0:2],
                    scratch2[0:2, 0:128],
                    start=True,
                    stop=True,
                )

            # DMA issue plan: sync HWDGE ring carries the critical path
            # (first xd half, per-step weights, second xd half);
            # prefetchables (identity, bias, xe, waves 1-3) go on the scalar
            # engine's separate ring.
            w_sb = singles.tile([128, NSTEP, 1024], MM_DT)
            ident_sb = singles.tile([128, 128], mybir.dt.float32r)
            bias_sb = singles.tile([K, 1], f32)
            out_sb = singles.tile([K, TLOC], f32)
            xds, xes = [], []
            sc_dmas = []
            sc_dmas.append(nc.scalar.dma_start(out=ident_sb, in_=ident[:, :]))
            sc_dmas.append(nc.scalar.dma_start(out=bias_sb, in_=biasc[:, :]))
            for w in range(NW):
                base = w * WCOLS
                # xd: rows 0-63 = xin shifts (j even), rows 64-127 = xin
                # shifted one further (j odd).  xe: rows 0-63 = xin shift 8,
                # row 64 = ones (bias row).
                xd = xpool.tile([128, XDW], MM_DT, name="xd")
                xe = xpool.tile([C + 1, WCOLS], MM_DT, name="xe")
                if w == 0:
                    nc.sync.dma_start(out=xd[:, 0:1036], in_=xin[:, 0:1036])
                    for s in range(NSTEP):
                        nc.sync.dma_start(
                            out=w_sb[:, s, :], in_=wts[:, s, :]
                        )
                    nc.sync.dma_start(out=xd[:, 1036:XDW], in_=xin[:, 1036:XDW])
                    sc_dmas.append(
                        nc.scalar.dma_start(
                            out=xe[0:C, :],
                            in_=xin[0:C, base + AR : base + AR + WCOLS],
                        )
                    )
                    sc_dmas.append(
                        nc.scalar.dma_start(out=xe[C : C + 1, :], in_=onesd[:, :])
                    )
                elif w == 1:
                    sc_dmas.append(
                        nc.scalar.dma_start(out=xd, in_=xin[:, base : base + XDW])
                    )
                    sc_dmas.append(
                        nc.scalar.dma_start(
                            out=xe[0:C, :],
                            in_=xin[0:C, base + AR : base + AR + WCOLS],
                        )
                    )
                    sc_dmas.append(
                        nc.scalar.dma_start(out=xe[C : C + 1, :], in_=onesd[:, :])
                    )
                xds.append(xd)
                xes.append(xe)

            def load_wave_inputs(w):
                # waves 2-3 load lazily (two waves ahead) so the prefetch
                # doesn't flood the DMA fabric while wave 0 computes
                base = w * WCOLS
                nc.scalar.dma_start(out=xds[w], in_=xin[:, base : base + XDW])
                nc.scalar.dma_start(
                    out=xes[w][0:C, :],
                    in_=xin[0:C, base + AR : base + AR + WCOLS],
                )
                nc.scalar.dma_start(out=xes[w][C : C + 1, :], in_=onesd[:, :])
            # DVE observer for the bias DMA (TS struct fits one wait)
            dve_scratch = singles.tile([K, 1], f32)
            nc.vector.tensor_copy(dve_scratch, bias_sb)

            first_sq = [True]

            def chunk_tail(w, off, psh):
                base = w * WCOLS
                m_sb = mpool.tile([128, K], mybir.dt.float32r, name="m_sb")
                for h in range(NH):
                    sq = sqpool.tile([128, 512], SQ_DT, name="sq", tag="sq")
                    sq_i = nc.scalar.activation(
                        sq, psh[h], mybir.ActivationFunctionType.Square
                    )
                    if first_sq[0]:
                        # the Act sequencer must issue every prefetch DMA
                        # before its first square, else a square that
                        # transitively gates one of those DMAs deadlocks
                        while sc_dmas:
                            add_dep_helper(sq_i.ins, sc_dmas.pop().ins, sync=False)
                        first_sq[0] = False
                    with nc.allow_low_precision(
                        reason="float32r shares float32 bits; r-mode only "
                        "affects the PE multiply path"
                    ):
                        nc.vector.tensor_reduce(
                            out=m_sb[:, 8 * h : 8 * h + 8],
                            in_=sq.rearrange("p (g c) -> p g c", g=8),
                            axis=mybir.AxisListType.X,
                            op=mybir.AluOpType.add,
                        )
                mt = mt_ps.tile([K, TC], mybir.dt.float32r, name="mt")
                _flush(nc.tensor.transpose(mt, m_sb, ident_sb))
                nc.vector.tensor_scalar(
                    out=out_sb[:, base + off : base + off + TC],
                    in0=mt[0:K, :],
                    scalar1=-0.5,
                    scalar2=bias_sb,
                    op0=mybir.AluOpType.mult,
                    op1=mybir.AluOpType.add,
                )

            def conv_lhsT(xd, xe, off, s):
                if s < 4:
                    return xd[:, off + 2 * s : off + 2 * s + TC]
                return xe[0 : C + 1, off : off + TC]

            def conv_rhs(s, h):
                if s < 4:
                    return w_sb[:, s, 512 * h : 512 * h + 512]
                return w_sb[0 : C + 1, s, 512 * h : 512 * h + 512]

            for w in range(NW):
                base = w * WCOLS
                xd = xds[w]
                xe = xes[w]
                if w + 2 < NW:
                    load_wave_inputs(w + 2)
                if w == 0:
                    # first four chunks pairwise s-major: the weight steps
                    # arrive one-by-one on the sync ring, so advance both
                    # chunks per step instead of stalling per chunk
                    for pair in ((0, 1), (2, 3)):
                        pshp = {
                            (c, h): conv_ps.tile(
                                [128, 512], f32, name=f"ps{c}{h}", tag="ps"
                            )
                            for c in pair
                            for h in range(NH)
                        }
                        for s in range(NSTEP):
                            if pair[0] == 0 and s == 0:
                                pe_observe(ident_sb[:, 0:2])
                                pe_observe(xd[:, 0:2])
                            if pair[0] == 0 and s == 4:
                                pe_observe(xe[0:C, 0:2])
                                pe_observe(xe[C : C + 1, 0:2])
                            for c in pair:
                                lhsT = conv_lhsT(xd, xe, c * TC, s)
                                for h in range(NH):
                                    pe_matmul(
                                        pshp[c, h],
                                        lhsT,
                                        conv_rhs(s, h),
                                        start=(s == 0),
                                        stop=(s == 4),
                                    )
                        for c in pair:
                            chunk_tail(w, c * TC, [pshp[c, h] for h in range(NH)])
                    start_tcl = 4
                else:
                    start_tcl = 0
                for tcl in range(start_tcl, WAVE):
                    off = tcl * TC
                    psh = [
                        conv_ps.tile([128, 512], f32, name=f"ps{h}", tag="ps")
                        for h in range(NH)
                    ]
                    for s in range(NSTEP):
                        if w > 0 and tcl == 0 and s == 0:
                            pe_observe(xd[:, 0:2])
                        if w > 0 and tcl == 0 and s == 4:
                            # lazily: s0-s3 must not stall on the xe loads
                            pe_observe(xe[0:C, 0:2])
                            pe_observe(xe[C : C + 1, 0:2])
                        lhsT = conv_lhsT(xd, xe, off, s)
                        for h in range(NH):
                            mm_i = pe_matmul(
                                psh[h],
                                lhsT,
                                conv_rhs(s, h),
                                start=(s == 0),
                                stop=(s == 4),
                            )
                    if tcl == WAVE - 2:
                        obs_after[0] = mm_i
                    chunk_tail(w, off, psh)
                if w < NW - 1:
                    nc.sync.dma_start(
                        out=out[:, base : base + WCOLS],
                        in_=out_sb[:, base : base + WCOLS],
                    )
                else:
                    # last wave: quarter DMAs so the final store is tiny
                    for q in range(4):
                        uq = base + q * (WCOLS // 4)
                        nc.sync.dma_start(
                            out=out[:, uq : uq + WCOLS // 4],
                            in_=out_sb[:, uq : uq + WCOLS // 4],
                        )
    nc.compile()
    return nc


def _prep_host(W, b, Sigma):
    """Fold L^{-1} into conv weights; pack moving-operand tiles, constants."""
    W64 = W.astype(np.float64)
    b64 = b.astype(np.float64)
    S64 = Sigma.astype(np.float64)
    L = np.linalg.cholesky(S64)
    Li = np.linalg.inv(L)                       # [K, C, C] lower-triangular inv
    logdet = 2.0 * np.sum(np.log(np.diagonal(L, axis1=1, axis2=2)), axis=1)
    W2 = np.einsum("kdc,kcij->kdij", Li, W64)   # [K, C(d), C(ci), 9]
    b2 = np.einsum("kdc,kc->kd", Li, b64)       # [K, C]

    # moving operand: w_np[r, s, 512*(k//8) + 64*(k%8) + d]
    #   s<4:  r = ci + 64*joff -> W2[k, d, ci, 2s+joff]
    #   s==4: r<64 -> W2[k, d, r, 8]; r==64 -> b2[k, d]; else 0
    w_np = np.zeros((128, NSTEP, 1024), np.float32)
    for s in range(4):
        # [ci + 64*joff, (k, d)]
        sub = W2[:, :, :, 2 * s : 2 * s + 2]        # [k, d, ci, joff]
        w_np[:, s, :] = np.transpose(sub, (3, 2, 0, 1)).reshape(128, 1024)
    w_np[0:C, 4, :] = np.transpose(W2[:, :, :, 8], (2, 0, 1)).reshape(C, 1024)
    w_np[C, 4, :] = b2.reshape(1024)

    const = C * np.log(2.0 * np.pi) + logdet
    bias_np = (-0.5 * const).astype(np.float32).reshape(K, 1)
    return w_np, bias_np


def _run(x, W, b, Sigma, trace=False):
    x = np.asarray(x, np.float32)
    W = np.asarray(W, np.float32)
    b = np.asarray(b, np.float32)
    Sigma = np.asarray(Sigma, np.float32)
    if "nc" not in _CACHE:
        _CACHE["nc"] = _build_program()
    nc = _CACHE["nc"]
    w_np, bias_np = _prep_host(W, b, Sigma)

    # left causal pad (AR) plus one right pad col so the shifted copy of the
    # last core's slice stays in bounds
    xpad = np.pad(np.asarray(x, np.float32)[0], ((0, 0), (AR, 1)))  # [C, T+9]
    in_maps = []
    for i in range(NCORES):
        lo = xpad[:, TLOC * i : TLOC * i + TLOC + AR]
        hi = xpad[:, TLOC * i + 1 : TLOC * i + TLOC + AR + 1]
        in_maps.append(
            {
                "xin": np.ascontiguousarray(
                    np.concatenate([lo, hi], axis=0).astype(_MM_NP)
                ),
                "wts": w_np.astype(_MM_NP),
                "ident": np.eye(128, dtype=np.float32),
                "biasc": bias_np,
                "onesd": np.ones((1, WCOLS), _MM_NP),
            }
        )
    res = run_bass_kernel_spmd(
        nc, in_maps, core_ids=list(range(NCORES)), trace=trace
    )
    outs = [res.results[i]["out"] for i in range(NCORES)]
    full = np.concatenate(outs, axis=1)[None]   # [1, K, T]
    return full.astype(np.float32), res


def kernel(x, W, b, Sigma):
    out, _ = _run(x, W, b, Sigma, trace=bool(int(os.environ.get("BASS_TRACE", "0"))))
    return out



# revision 4
# speedup vs baseline: 1.0751x; 1.0751x over previous
"""Trainium2 Bass kernel for nn_Autoregression (16-state AR whitening log-prob).

Math: reference computes log_prob[b,k,t] = -0.5*(C*log(2pi) + logdet(Sigma_k)
+ es_k(t)^T Sigma_k^{-1} es_k(t)) with es = causal_conv(x, W, b).  Since
Sigma^{-1} = L^{-T} L^{-1} and es is affine in x, fold L^{-1} into the conv:
W2 = L^{-1} W, b2 = L^{-1} b, then mahalanobis = sum_c conv(x; W2, b2)^2.

fp8 DoubleRow version: conv matmuls run in fp8e4 (e4m3) with
perf_mode=DoubleRow, which packs 2 contraction rows per PE cell (virtual
128x256 array).  Contraction of 577 rows (9 taps x 64 cin + bias) per output
chunk is packed as 2 full DR steps of 256 virtual rows (taps 0-7; partition
p = (cin, g), pair slot i covers tap 4s+2g+i) plus one DR leftover step
(tap 8 as channel-pairs on 32 partitions + a ones/bias row).  PSUM chunk is
[128 t, 512 (8 states x 64 ch)] x 2 halves.  ACT squares PSUM -> bf16 SBUF
with the free scale folding in 1/(Sx*Sw*sqrt(2)); DVE does the per-state
segmented reduce with negate ([128,8,64] -> -[128,8]); a small PE transpose
flips [128 t, 16 k] -> [16 k, 128 t] batched 4 chunks per PSUM tile; ACT
adds the per-state constant; DMA out.
"""

import math
import os

import numpy as np
import ml_dtypes

import concourse.bass as bass
import concourse.bacc as bacc_mod
import concourse.mybir as mybir
import concourse.tile as tile
from concourse.bass_utils import run_bass_kernel_spmd
from concourse.tile_rust import add_dep_helper

K = 16          # states
C = 64          # channels
T = 65536       # time
AR = 8          # ar order (kernel size AR+1)
NCORES = 8
TLOC = T // NCORES          # 8192 outputs per core
TC = 128                    # outputs per chunk (matmul M)
WAVE = 16                   # chunks per wave (input tile granularity)
WCOLS = TC * WAVE           # 2048 outputs per wave
NW = TLOC // WCOLS          # waves per core
NH = 2                      # psum halves (states 0-7, 8-15)
NS = 2                      # full DoubleRow contraction steps (taps 0-7)
MTGRP = 4                   # chunks batched per [16, 512] transpose psum

# DoubleRow LDWEIGHTS requires the pair-region byte stride % 16 == 0
XWW = WCOLS + 16            # xq wave-tile region width (max col 2051 used)
XQW = (NW - 1) * WCOLS + XWW  # xq dram region width

FP8 = mybir.dt.float8e4
SQ_DT = mybir.dt.bfloat16   # squares dtype
DR = mybir.MatmulPerfMode.DoubleRow

SX = 16.0                   # x scale into fp8
SW = 64.0                   # weight scale into fp8
ACT_SCALE = 1.0 / (SX * SW * math.sqrt(2.0))

_FP8_NP = ml_dtypes.float8_e4m3
FP8_MAX = 240.0

_CACHE: dict = {}


def _build_program():
    nc = bacc_mod.Bacc()
    f32 = mybir.dt.float32

    # xq rows p=(c,g): 2 regions i: x[c, t0-8 + a + 2g + i] * SX (taps 0-7)
    xq = nc.declare_dram_parameter("xq", [128, 2, XQW], FP8, isOutput=False)
    # xe rows p<32: 2 regions i: x[2p+i, t0 + a] * SX (tap 8);
    # row 32: (ones, zeros); rows 33-63: zeros
    xe = nc.declare_dram_parameter("xe", [64, 2, TLOC], FP8, isOutput=False)
    # wts[p, i, s, n] = SW * W2[k(n), d(n), c(p), 4s + 2g(p) + i]
    wts = nc.declare_dram_parameter("wts", [128, 2, NS, 1024], FP8, isOutput=False)
    # web[p, i, n] = SW * W2[k, d, 2p+i, 8] (p<32); row 32 i=0: SX*SW*b2
    web = nc.declare_dram_parameter("web", [64, 2, 1024], FP8, isOutput=False)
    ident = nc.declare_dram_parameter("ident", [128, 128], mybir.dt.float32r, isOutput=False)
    biasc = nc.declare_dram_parameter("biasc", [K, 1], f32, isOutput=False)
    out = nc.declare_dram_parameter("out", [K, TLOC], f32, isOutput=True)

    with tile.TileContext(nc) as tc:
        with (
            tc.tile_pool(name="singles", bufs=1) as singles,
            # one slot per wave: input DMAs never wait (no slot WAR/WAW)
            tc.tile_pool(name="xpool", bufs=NW) as xpool,
            tc.tile_pool(name="sqpool", bufs=12) as sqpool,
            tc.tile_pool(name="mpool", bufs=6) as mpool,
            tc.tile_pool(name="conv_ps", bufs=5, space="PSUM") as conv_ps,
            tc.tile_pool(name="mt_ps", bufs=2, space="PSUM") as mt_ps,
            tc.tile_pool(name="obs_ps", bufs=1, space="PSUM") as obs_ps,
        ):
            # Matmuls must never be the first PE instruction to observe more
            # than one producer semaphore (1-wait ISA slots; bacc's event-sem
            # legalization costs sequencer time).  pe_observe() emits a tiny
            # 2x2 "reader" matmul whose operands come from a single
            # producer's tile; ordering edges pin readers ahead of the next
            # real matmul.
            scratch = obs_ps.tile([2, 128], f32)
            scratch2 = singles.tile([2, 128], SQ_DT)
            nc.vector.memset(scratch2, 0.0)
            pending = []
            obs_after = [None]

            def pe_observe(col):
                i = nc.tensor.matmul(
                    scratch[0:2, 0:2], col, col, start=True, stop=True
                )
                if obs_after[0] is not None:
                    # not earlier than late in the previous wave, or the PE
                    # FIFO head-of-line blocks on a DMA that hasn't landed
                    add_dep_helper(i.ins, obs_after[0].ins, sync=False)
                pending.append(i)

            def _flush(i):
                while pending:
                    add_dep_helper(i.ins, pending.pop().ins, sync=False)
                return i

            def pe_matmul(*args, **kw):
                return _flush(nc.tensor.matmul(*args, **kw))

            # dep-free warmup matmuls: keep the PE busy through the initial
            # input DMAs so HAM un-throttles before real work
            for _ in range(35):
                nc.tensor.matmul(
                    scratch[0:2, 0:128],
                    scratch2[0:2, 0:2],
                    scratch2[0:2, 0:128],
                    start=True,
                    stop=True,
                )

            # DMA issue plan: sync HWDGE ring carries the critical path
            # (first xq piece, weights, rest of xq); prefetchables
            # (identity, bias, xe/web, waves 1+) go on the scalar ring.
            w_sb = singles.tile([128, 2, NS, 1024], FP8)
            web_sb = singles.tile([64, 2, 1024], FP8)
            ident_sb = singles.tile([128, 128], mybir.dt.float32r)
            bias_sb = singles.tile([K, 1], f32)
            out_sb = singles.tile([K, TLOC], f32)
            xqs, xes = [], []
            sc_dmas = []
            sc_dmas.append(nc.scalar.dma_start(out=ident_sb, in_=ident[:, :]))
            sc_dmas.append(nc.scalar.dma_start(out=bias_sb, in_=biasc[:, :]))
            for w in range(NW):
                base = w * WCOLS
                xq_w = xpool.tile([128, 2, XWW], FP8, name="xq_w")
                xe_w = xpool.tile([64, 2, WCOLS], FP8, name="xe_w")
                if w == 0:
                    # first piece covers chunks 0-1 (cols < 260)
                    nc.sync.dma_start(out=xq_w[:, :, 0:264], in_=xq[:, :, 0:264])
                    nc.sync.dma_start(out=w_sb, in_=wts[:, :, :, :])
                    nc.sync.dma_start(
                        out=xq_w[:, :, 264:XWW], in_=xq[:, :, 264:XWW]
                    )
                    sc_dmas.append(
                        nc.scalar.dma_start(
                            out=xe_w, in_=xe[:, :, base : base + WCOLS]
                        )
                    )
                    sc_dmas.append(nc.scalar.dma_start(out=web_sb, in_=web[:, :, :]))
                elif w == 1:
                    sc_dmas.append(
                        nc.scalar.dma_start(
                            out=xq_w, in_=xq[:, :, base : base + XWW]
                        )
                    )
                    sc_dmas.append(
                        nc.scalar.dma_start(
                            out=xe_w, in_=xe[:, :, base : base + WCOLS]
                        )
                    )
                xqs.append(xq_w)
                xes.append(xe_w)

            def load_wave_inputs(w):
                # waves 2-3 load lazily (two waves ahead) so the prefetch
                # doesn't flood the DMA fabric while wave 0 computes
                base = w * WCOLS
                nc.scalar.dma_start(out=xqs[w], in_=xq[:, :, base : base + XWW])
                nc.scalar.dma_start(out=xes[w], in_=xe[:, :, base : base + WCOLS])

            # DVE observer for the bias DMA (TS struct fits one wait)
            dve_scratch = singles.tile([K, 1], f32)
            nc.vector.tensor_copy(dve_scratch, bias_sb)

            first_sq = [True]
            mts = [None]

            def chunk_tail(w, cglob, off, psh):
                base = w * WCOLS
                m_sb = mpool.tile([128, K], mybir.dt.float32r, name="m_sb")
                for h in range(NH):
                    sq = sqpool.tile([128, 512], SQ_DT, name="sq", tag="sq")
                    sq_i = nc.scalar.activation(
                        sq,
                        psh[h],
                        mybir.ActivationFunctionType.Square,
                        scale=ACT_SCALE,
                    )
                    if first_sq[0]:
                        # the Act sequencer must issue every prefetch DMA
                        # before its first square, else a square that
                        # transitively gates one of those DMAs deadlocks
                        while sc_dmas:
                            add_dep_helper(sq_i.ins, sc_dmas.pop().ins, sync=False)
                        first_sq[0] = False
                    with nc.allow_low_precision(
                        reason="float32r shares float32 bits; r-mode only "
                        "affects the PE multiply path"
                    ):
                        nc.vector.tensor_reduce(
                            out=m_sb[:, 8 * h : 8 * h + 8],
                            in_=sq.rearrange("p (g c) -> p g c", g=8),
                            axis=mybir.AxisListType.X,
                            op=mybir.AluOpType.add,
                            negate=True,
                        )
                g = cglob % MTGRP
                if g == 0:
                    mts[0] = mt_ps.tile([K, MTGRP * TC], mybir.dt.float32r, name="mt")
                mt = mts[0]
                _flush(
                    nc.tensor.transpose(mt[:, g * TC : (g + 1) * TC], m_sb, ident_sb)
                )
                if g == MTGRP - 1:
                    gbase = (cglob - g) * TC
                    # out = -m/2 + (-0.5*(Dlog2pi + logdet))  on ACT
                    nc.scalar.activation(
                        out_sb[:, gbase : gbase + MTGRP * TC],
                        mt[0:K, :],
                        mybir.ActivationFunctionType.Identity,
                        bias=bias_sb,
                        scale=1.0,
                    )

            for w in range(NW):
                base = w * WCOLS
                xq_w = xqs[w]
                xe_w = xes[w]
                if w + 2 < NW:
                    load_wave_inputs(w + 2)
                for tcl in range(WAVE):
                    off = tcl * TC
                    cglob = w * WAVE + tcl
                    psh = [
                        conv_ps.tile([128, 512], mybir.dt.float32, name=f"ps{h}", tag="ps")
                        for h in range(NH)
                    ]
                    if tcl == 0:
                        if w == 0:
                            pe_observe(ident_sb[:, 0:2])
                        pe_observe(xq_w[0:2, 0, 0:2])
                    for s in range(NS):
                        lhsT = xq_w[:, :, off + 4 * s : off + 4 * s + TC]
                        for h in range(NH):
                            pe_matmul(
                                psh[h],
                                lhsT,
                                w_sb[:, :, s, 512 * h : 512 * h + 512],
                                start=(s == 0),
                                stop=False,
                                perf_mode=DR,
                            )
                    if tcl == 0:
                        # lazily: s0-s1 must not stall on the xe/web loads
                        pe_observe(xe_w[0:2, 0, 0:2])
                        pe_observe(web_sb[0:2, 0, 0:2])
                    lhsT_e = xe_w[:, :, off : off + TC]
                    for h in range(NH):
                        mm_i = pe_matmul(
                            psh[h],
                            lhsT_e,
                            web_sb[:, :, 512 * h : 512 * h + 512],
                            start=False,
                            stop=True,
                            perf_mode=DR,
                        )
                    if tcl == WAVE - 2:
                        obs_after[0] = mm_i
                    chunk_tail(w, cglob, off, psh)
                if w < NW - 1:
                    nc.sync.dma_start(
                        out=out[:, base : base + WCOLS],
                        in_=out_sb[:, base : base + WCOLS],
                    )
                else:
                    # last wave: quarter DMAs so the final store is tiny
                    for q in range(4):
                        uq = base + q * (WCOLS // 4)
                        nc.sync.dma_start(
                            out=out[:, uq : uq + WCOLS // 4],
                            in_=out_sb[:, uq : uq + WCOLS // 4],
                        )
    nc.compile()
    return nc


def _prep_host(W, b, Sigma):
    """Fold L^{-1} into conv weights; pack fp8 DoubleRow tiles, constants."""
    W64 = W.astype(np.float64)
    b64 = b.astype(np.float64)
    S64 = Sigma.astype(np.float64)
    L = np.linalg.cholesky(S64)
    Li = np.linalg.inv(L)                       # [K, C, C] lower-triangular inv
    logdet = 2.0 * np.sum(np.log(np.diagonal(L, axis1=1, axis2=2)), axis=1)
    W2 = np.einsum("kdc,kcij->kdij", Li, W64)   # [K, C(d), C(ci), 9]
    b2 = np.einsum("kdc,kc->kd", Li, b64)       # [K, C]

    def q8(v):
        return np.clip(v, -FP8_MAX, FP8_MAX).astype(_FP8_NP)

    # weight column layout: n = 512*(k//8) + 64*(k%8) + d
    W2n = np.transpose(W2, (0, 1, 3, 2)).reshape(K * C, 9, C)  # [(k,d), j, c]
    W2n = W2n.reshape(2, 512, 9, C)                            # [h, n', j, c]

    # wts[p, i, s, n] = SW * W2[k(n), d(n), c(p), 4s + 2g(p) + i]
    wts_np = np.zeros((128, 2, NS, 1024), np.float64)
    for g in range(2):
        for i in range(2):
            for s in range(NS):
                j = 4 * s + 2 * g + i
                # [c, (h, n')] for tap j
                wj = np.transpose(W2n[:, :, j, :], (2, 0, 1)).reshape(C, 1024)
                wts_np[64 * g : 64 * g + 64, i, s, :] = SW * wj
    # web[p, i, n] = SW * W2[k, d, 2p+i, 8] (p<32); row 32 i=0: SX*SW*b2
    web_np = np.zeros((64, 2, 1024), np.float64)
    w8 = np.transpose(W2n[:, :, 8, :], (2, 0, 1)).reshape(C, 1024)  # [c, n]
    web_np[0:32, 0, :] = SW * w8[0::2, :]
    web_np[0:32, 1, :] = SW * w8[1::2, :]
    web_np[32, 0, :] = SX * SW * b2.reshape(2, 8, 64).reshape(1024)

    const = C * np.log(2.0 * np.pi) + logdet
    bias_np = (-0.5 * const).astype(np.float32).reshape(K, 1)
    return q8(wts_np), q8(web_np), bias_np


def _run(x, W, b, Sigma, trace=False):
    x = np.asarray(x, np.float32)
    W = np.asarray(W, np.float32)
    b = np.asarray(b, np.float32)
    Sigma = np.asarray(Sigma, np.float32)
    if "nc" not in _CACHE:
        _CACHE["nc"] = _build_program()
    nc = _CACHE["nc"]
    wts_np, web_np, bias_np = _prep_host(W, b, Sigma)

    # causal left pad (AR) plus right pad so shifted copies stay in bounds
    xp = np.pad(x[0].astype(np.float64), ((0, 0), (AR, 24)))       # [C, T+32]
    xp8 = np.clip(SX * xp, -FP8_MAX, FP8_MAX).astype(_FP8_NP)
    ident_np = np.eye(128, dtype=np.float32)
    in_maps = []
    for ci in range(NCORES):
        t0 = ci * TLOC
        # xq[p, i, a] = xp8[c, t0 + a + 2g + i]
        xq_np = np.zeros((128, 2, XQW), _FP8_NP)
        for g in range(2):
            for i in range(2):
                sh = 2 * g + i
                xq_np[64 * g : 64 * g + 64, i, :] = xp8[:, t0 + sh : t0 + sh + XQW]
        # xe[p, i, a] = xp8[2p+i, t0 + 8 + a] (p<32); row 32 = (1, 0)
        xe_np = np.zeros((64, 2, TLOC), _FP8_NP)
        xe_np[0:32, 0, :] = xp8[0::2, t0 + 8 : t0 + 8 + TLOC]
        xe_np[0:32, 1, :] = xp8[1::2, t0 + 8 : t0 + 8 + TLOC]
        xe_np[32, 0, :] = _FP8_NP(1.0)
        in_maps.append(
            {
                "xq": xq_np,
                "xe": xe_np,
                "wts": wts_np,
                "web": web_np,
                "ident": ident_np,
                "biasc": bias_np,
            }
        )
    res = run_bass_kernel_spmd(
        nc, in_maps, core_ids=list(range(NCORES)), trace=trace
    )
    outs = [res.results[i]["out"] for i in range(NCORES)]
    full = np.concatenate(outs, axis=1)[None]   # [1, K, T]
    return full.astype(np.float32), res


def kernel(x, W, b, Sigma):
    out, _ = _run(x, W, b, Sigma, trace=bool(int(os.environ.get("BASS_TRACE", "0"))))
    return out


# revision 10
# speedup vs baseline: 1.0889x; 1.0129x over previous
"""Trainium2 Bass kernel for nn_Autoregression (16-state AR whitening log-prob).

Math: reference computes log_prob[b,k,t] = -0.5*(C*log(2pi) + logdet(Sigma_k)
+ es_k(t)^T Sigma_k^{-1} es_k(t)) with es = causal_conv(x, W, b).  Since
Sigma^{-1} = L^{-T} L^{-1} and es is affine in x, fold L^{-1} into the conv:
W2 = L^{-1} W, b2 = L^{-1} b, then mahalanobis = sum_c conv(x; W2, b2)^2.

fp8 DoubleRow version: conv matmuls run in fp8e4 (e4m3) with
perf_mode=DoubleRow, which packs 2 contraction rows per PE cell (virtual
128x256 array).  Contraction of 577 rows (9 taps x 64 cin + bias) per output
chunk is packed as 2 full DR steps of 256 virtual rows (taps 0-7; partition
p = (cin, g), pair slot i covers tap 4s+2g+i) plus one DR leftover step
(tap 8 as channel-pairs on 32 partitions + a ones/bias row).  PSUM chunk is
[128 t, 512 (8 states x 64 ch)] x 2 halves.  ACT squares PSUM -> bf16 SBUF
with the free scale folding in 1/(Sx*Sw*sqrt(2)); DVE does the per-state
segmented reduce with negate ([128,8,64] -> -[128,8]); a small PE transpose
flips [128 t, 16 k] -> [16 k, 128 t] batched 4 chunks per PSUM tile; ACT
adds the per-state constant; DMA out.
"""

import math
import os

import numpy as np
import ml_dtypes

import concourse.bass as bass
import concourse.bacc as bacc_mod
import concourse.mybir as mybir
import concourse.tile as tile
from concourse.bass_utils import run_bass_kernel_spmd
from concourse.tile_rust import add_dep_helper

K = 16          # states
C = 64          # channels
T = 65536       # time
AR = 8          # ar order (kernel size AR+1)
NCORES = 8
TLOC = T // NCORES          # 8192 outputs per core
TC = 128                    # outputs per chunk (matmul M)
WAVE = 16                   # chunks per wave (input tile granularity)
WCOLS = TC * WAVE           # 2048 outputs per wave
NW = TLOC // WCOLS          # waves per core
NH = 2                      # psum halves (states 0-7, 8-15)
NS = 2                      # full DoubleRow contraction steps (taps 0-7)
MTGRP = 4                   # chunks batched per [16, 512] transpose psum

# DoubleRow LDWEIGHTS requires the pair-region byte stride % 16 == 0
XWW = WCOLS + 16            # xq wave-tile region width (max col 2051 used)
XQW = (NW - 1) * WCOLS + XWW  # xq dram region width

FP8 = mybir.dt.float8e4
SQ_DT = mybir.dt.bfloat16   # squares dtype
DR = mybir.MatmulPerfMode.DoubleRow

SX = 16.0                   # x scale into fp8
SW = 64.0                   # weight scale into fp8
ACT_SCALE = 1.0 / (SX * SW * math.sqrt(2.0))

_FP8_NP = ml_dtypes.float8_e4m3
FP8_MAX = 240.0

_CACHE: dict = {}


def _build_program():
    nc = bacc_mod.Bacc()
    f32 = mybir.dt.float32

    # xq rows p=(c,g): 2 regions i: x[c, t0-8 + a + 2g + i] * SX (taps 0-7)
    xq = nc.declare_dram_parameter("xq", [128, 2, XQW], FP8, isOutput=False)
    # xe rows p<32: 2 regions i: x[2p+i, t0 + a] * SX (tap 8);
    # row 32: (ones, zeros); rows 33-63: zeros
    xe = nc.declare_dram_parameter("xe", [64, 2, TLOC], FP8, isOutput=False)
    # moving operands store DoubleRow pairs interleaved (contiguous byte
    # pairs stream at full rate; split regions force 2 fetches/cycle)
    # wts[p, s, n, i] = SW * W2[k(n), d(n), c(p), 4s + 2g(p) + i]
    wts = nc.declare_dram_parameter("wts", [128, NS, 1024, 2], FP8, isOutput=False)
    # web[p, n, i] = SW * W2[k, d, 2p+i, 8] (p<32); row 32 i=0: SX*SW*b2
    web = nc.declare_dram_parameter("web", [64, 1024, 2], FP8, isOutput=False)
    ident = nc.declare_dram_parameter("ident", [128, 128], mybir.dt.float32r, isOutput=False)
    biasc = nc.declare_dram_parameter("biasc", [K, 1], f32, isOutput=False)
    out = nc.declare_dram_parameter("out", [K, TLOC], f32, isOutput=True)

    with tile.TileContext(nc) as tc:
        with (
            tc.tile_pool(name="singles", bufs=1) as singles,
            # one slot per wave: input DMAs never wait (no slot WAR/WAW)
            tc.tile_pool(name="xpool", bufs=NW) as xpool,
            tc.tile_pool(name="sqpool", bufs=12) as sqpool,
            tc.tile_pool(name="mpool", bufs=6) as mpool,
            tc.tile_pool(name="conv_ps", bufs=5, space="PSUM") as conv_ps,
            tc.tile_pool(name="mt_ps", bufs=2, space="PSUM") as mt_ps,
            tc.tile_pool(name="obs_ps", bufs=1, space="PSUM") as obs_ps,
        ):
            # Matmuls must never be the first PE instruction to observe more
            # than one producer semaphore (1-wait ISA slots; bacc's event-sem
            # legalization costs sequencer time).  pe_observe() emits a tiny
            # 2x2 "reader" matmul whose operands come from a single
            # producer's tile; ordering edges pin readers ahead of the next
            # real matmul.
            scratch = obs_ps.tile([2, 128], f32)
            scratch2 = singles.tile([2, 128], SQ_DT)
            nc.vector.memset(scratch2, 0.0)
            pending = []
            obs_after = [None]

            def pe_observe(col):
                i = nc.tensor.matmul(
                    scratch[0:2, 0:2], col, col, start=True, stop=True
                )
                if obs_after[0] is not None:
                    # not earlier than late in the previous wave, or the PE
                    # FIFO head-of-line blocks on a DMA that hasn't landed
                    add_dep_helper(i.ins, obs_after[0].ins, sync=False)
                pending.append(i)

            def _flush(i):
                while pending:
                    add_dep_helper(i.ins, pending.pop().ins, sync=False)
                return i

            def pe_matmul(*args, **kw):
                return _flush(nc.tensor.matmul(*args, **kw))

            # dep-free warmup matmuls: keep the PE busy through the initial
            # input DMAs so HAM un-throttles before real work
            for _ in range(35):
                nc.tensor.matmul(
                    scratch[0:2, 0:128],
                    scratch2[0:2, 0:2],
                    scratch2[0:2, 0:128],
                    start=True,
                    stop=True,
                )

            # DMA issue plan: sync HWDGE ring carries the critical path
            # (first xq piece, weights, rest of xq); prefetchables
            # (identity, bias, xe/web, waves 1+) go on the scalar ring.
            w_sb = singles.tile([128, NS, 1024, 2], FP8)
            web_sb = singles.tile([64, 1024, 2], FP8)
            ident_sb = singles.tile([128, 128], mybir.dt.float32r)
            bias_sb = singles.tile([K, 1], f32)
            out_sb = singles.tile([K, TLOC], f32)
            xqs, xes = [], []
            sc_dmas = []
            sc_dmas.append(nc.scalar.dma_start(out=ident_sb, in_=ident[:, :]))
            sc_dmas.append(nc.scalar.dma_start(out=bias_sb, in_=biasc[:, :]))
            for w in range(NW):
                base = w * WCOLS
                xq_w = xpool.tile([128, 2, XWW], FP8, name="xq_w")
                xe_w = xpool.tile([64, 2, WCOLS], FP8, name="xe_w")
                if w == 0:
                    # first piece covers chunks 0-1 (cols < 260)
                    nc.sync.dma_start(out=xq_w[:, :, 0:264], in_=xq[:, :, 0:264])
                    nc.sync.dma_start(out=w_sb, in_=wts[:, :, :, :])
                    w_mv = w_sb.rearrange("p s n i -> p s i n")
                    web_mv = web_sb.rearrange("p n i -> p i n")
                    nc.sync.dma_start(
                        out=xq_w[:, :, 264:XWW], in_=xq[:, :, 264:XWW]
                    )
                    sc_dmas.append(
                        nc.scalar.dma_start(
                            out=xe_w, in_=xe[:, :, base : base + WCOLS]
                        )
                    )
                    sc_dmas.append(nc.scalar.dma_start(out=web_sb, in_=web[:, :, :]))
                elif w == 1:
                    sc_dmas.append(
                        nc.scalar.dma_start(
                            out=xq_w, in_=xq[:, :, base : base + XWW]
                        )
                    )
                    sc_dmas.append(
                        nc.scalar.dma_start(
                            out=xe_w, in_=xe[:, :, base : base + WCOLS]
                        )
                    )
                xqs.append(xq_w)
                xes.append(xe_w)

            def load_wave_inputs(w):
                # waves 2-3 load lazily (two waves ahead) so the prefetch
                # doesn't flood the DMA fabric while wave 0 computes
                base = w * WCOLS
                nc.scalar.dma_start(out=xqs[w], in_=xq[:, :, base : base + XWW])
                nc.scalar.dma_start(out=xes[w], in_=xe[:, :, base : base + WCOLS])

            # DVE observer for the bias DMA (TS struct fits one wait)
            dve_scratch = singles.tile([K, 1], f32)
            nc.vector.tensor_copy(dve_scratch, bias_sb)

            first_sq = [True]
            mts = [None]

            def chunk_tail(w, cglob, off, psh):
                base = w * WCOLS
                m_sb = mpool.tile([128, K], mybir.dt.float32r, name="m_sb")
                for h in range(NH):
                    sq = sqpool.tile([128, 512], SQ_DT, name="sq", tag="sq")
                    sq_i = nc.scalar.activation(
                        sq,
                        psh[h],
                        mybir.ActivationFunctionType.Square,
                        scale=ACT_SCALE,
                    )
                    if first_sq[0]:
                        # the Act sequencer must issue every prefetch DMA
                        # before its first square, else a square that
                        # transitively gates one of those DMAs deadlocks
                        while sc_dmas:
                            add_dep_helper(sq_i.ins, sc_dmas.pop().ins, sync=False)
                        first_sq[0] = False
                    with nc.allow_low_precision(
                        reason="float32r shares float32 bits; r-mode only "
                        "affects the PE multiply path"
                    ):
                        nc.vector.tensor_reduce(
                            out=m_sb[:, 8 * h : 8 * h + 8],
                            in_=sq.rearrange("p (g c) -> p g c", g=8),
                            axis=mybir.AxisListType.X,
                            op=mybir.AluOpType.add,
                            negate=True,
                        )
                g = cglob % MTGRP
                if g == 0:
                    mts[0] = mt_ps.tile([K, MTGRP * TC], mybir.dt.float32r, name="mt")
                mt = mts[0]
                _flush(
                    nc.tensor.transpose(mt[:, g * TC : (g + 1) * TC], m_sb, ident_sb)
                )
                if g == MTGRP - 1:
                    gbase = (cglob - g) * TC
                    # out = -m/2 + (-0.5*(Dlog2pi + logdet))  on ACT
                    nc.scalar.activation(
                        out_sb[:, gbase : gbase + MTGRP * TC],
                        mt[0:K, :],
                        mybir.ActivationFunctionType.Identity,
                        bias=bias_sb,
                        scale=1.0,
                    )

            for w in range(NW):
                base = w * WCOLS
                xq_w = xqs[w]
                xe_w = xes[w]
                if w + 2 < NW:
                    load_wave_inputs(w + 2)
                for tcl in range(WAVE):
                    off = tcl * TC
                    cglob = w * WAVE + tcl
                    psh = [
                        conv_ps.tile([128, 512], mybir.dt.float32, name=f"ps{h}", tag="ps")
                        for h in range(NH)
                    ]
                    if tcl == 0:
                        if w == 0:
                            pe_observe(ident_sb[:, 0:2])
                        pe_observe(xq_w[0:2, 0, 0:2])
                    for s in range(NS):
                        lhsT = xq_w[:, :, off + 4 * s : off + 4 * s + TC]
                        for h in range(NH):
                            pe_matmul(
                                psh[h],
                                lhsT,
                                w_mv[:, s, :, 512 * h : 512 * h + 512],
                                start=(s == 0),
                                stop=False,
                                perf_mode=DR,
                            )
                    if tcl == 0:
                        # lazily: s0-s1 must not stall on the xe/web loads
                        pe_observe(xe_w[0:2, 0, 0:2])
                        pe_observe(web_sb[0:2, 0, 0:2])
                    lhsT_e = xe_w[:, :, off : off + TC]
                    for h in range(NH):
                        mm_i = pe_matmul(
                            psh[h],
                            lhsT_e,
                            web_mv[:, :, 512 * h : 512 * h + 512],
                            start=False,
                            stop=True,
                            perf_mode=DR,
                        )
                    if tcl == WAVE - 2:
                        obs_after[0] = mm_i
                    chunk_tail(w, cglob, off, psh)
                if w < NW - 1:
                    nc.sync.dma_start(
                        out=out[:, base : base + WCOLS],
                        in_=out_sb[:, base : base + WCOLS],
                    )
                else:
                    # last wave: quarter DMAs so the final store is tiny
                    for q in range(4):
                        uq = base + q * (WCOLS // 4)
                        nc.sync.dma_start(
                            out=out[:, uq : uq + WCOLS // 4],
                            in_=out_sb[:, uq : uq + WCOLS // 4],
                        )
    nc.compile()
    return nc


def _prep_host(W, b, Sigma):
    """Fold L^{-1} into conv weights; pack fp8 DoubleRow tiles, constants."""
    W64 = W.astype(np.float64)
    b64 = b.astype(np.float64)
    S64 = Sigma.astype(np.float64)
    L = np.linalg.cholesky(S64)
    Li = np.linalg.inv(L)                       # [K, C, C] lower-triangular inv
    logdet = 2.0 * np.sum(np.log(np.diagonal(L, axis1=1, axis2=2)), axis=1)
    W2 = np.einsum("kdc,kcij->kdij", Li, W64)   # [K, C(d), C(ci), 9]
    b2 = np.einsum("kdc,kc->kd", Li, b64)       # [K, C]

    def q8(v):
        return np.clip(v, -FP8_MAX, FP8_MAX).astype(_FP8_NP)

    # weight column layout: n = 512*(k//8) + 64*(k%8) + d
    W2n = np.transpose(W2, (0, 1, 3, 2)).reshape(K * C, 9, C)  # [(k,d), j, c]
    W2n = W2n.reshape(2, 512, 9, C)                            # [h, n', j, c]

    # wts[p, i, s, n] = SW * W2[k(n), d(n), c(p), 4s + 2g(p) + i]
    wts_np = np.zeros((128, 2, NS, 1024), np.float64)
    for g in range(2):
        for i in range(2):
            for s in range(NS):
                j = 4 * s + 2 * g + i
                # [c, (h, n')] for tap j
                wj = np.transpose(W2n[:, :, j, :], (2, 0, 1)).reshape(C, 1024)
                wts_np[64 * g : 64 * g + 64, i, s, :] = SW * wj
    # web[p, i, n] = SW * W2[k, d, 2p+i, 8] (p<32); row 32 i=0: SX*SW*b2
    web_np = np.zeros((64, 2, 1024), np.float64)
    w8 = np.transpose(W2n[:, :, 8, :], (2, 0, 1)).reshape(C, 1024)  # [c, n]
    web_np[0:32, 0, :] = SW * w8[0::2, :]
    web_np[0:32, 1, :] = SW * w8[1::2, :]
    web_np[32, 0, :] = SX * SW * b2.reshape(2, 8, 64).reshape(1024)

    const = C * np.log(2.0 * np.pi) + logdet
    bias_np = (-0.5 * const).astype(np.float32).reshape(K, 1)
    # interleave DoubleRow pairs along the innermost byte
    wts_il = np.ascontiguousarray(np.transpose(wts_np, (0, 2, 3, 1)))
    web_il = np.ascontiguousarray(np.transpose(web_np, (0, 2, 1)))
    return q8(wts_il), q8(web_il), bias_np


def _run(x, W, b, Sigma, trace=False):
    x = np.asarray(x, np.float32)
    W = np.asarray(W, np.float32)
    b = np.asarray(b, np.float32)
    Sigma = np.asarray(Sigma, np.float32)
    if "nc" not in _CACHE:
        _CACHE["nc"] = _build_program()
    nc = _CACHE["nc"]
    wts_np, web_np, bias_np = _prep_host(W, b, Sigma)

    # causal left pad (AR) plus right pad so shifted copies stay in bounds
    xp = np.pad(x[0].astype(np.float64), ((0, 0), (AR, 24)))       # [C, T+32]
    xp8 = np.clip(SX * xp, -FP8_MAX, FP8_MAX).astype(_FP8_NP)
    ident_np = np.eye(128, dtype=np.float32)
    in_maps = []
    for ci in range(NCORES):
        t0 = ci * TLOC
        # xq[p, i, a] = xp8[c, t0 + a + 2g + i]
        xq_np = np.zeros((128, 2, XQW), _FP8_NP)
        for g in range(2):
            for i in range(2):
                sh = 2 * g + i
                xq_np[64 * g : 64 * g + 64, i, :] = xp8[:, t0 + sh : t0 + sh + XQW]
        # xe[p, i, a] = xp8[2p+i, t0 + 8 + a] (p<32); row 32 = (1, 0)
        xe_np = np.zeros((64, 2, TLOC), _FP8_NP)
        xe_np[0:32, 0, :] = xp8[0::2, t0 + 8 : t0 + 8 + TLOC]
        xe_np[0:32, 1, :] = xp8[1::2, t0 + 8 : t0 + 8 + TLOC]
        xe_np[32, 0, :] = _FP8_NP(1.0)
        in_maps.append(
            {
                "xq": xq_np,
                "xe": xe_np,
                "wts": wts_np,
                "web": web_np,
                "ident": ident_np,
                "biasc": bias_np,
            }
        )
    res = run_bass_kernel_spmd(
        nc, in_maps, core_ids=list(range(NCORES)), trace=trace
    )
    outs = [res.results[i]["out"] for i in range(NCORES)]
    full = np.concatenate(outs, axis=1)[None]   # [1, K, T]
    return full.astype(np.float32), res


def kernel(x, W, b, Sigma):
    out, _ = _run(x, W, b, Sigma, trace=bool(int(os.environ.get("BASS_TRACE", "0"))))
    return out


# revision 18
# speedup vs baseline: 1.1060x; 1.0157x over previous
"""Trainium2 Bass kernel for nn_Autoregression (16-state AR whitening log-prob).

Math: reference computes log_prob[b,k,t] = -0.5*(C*log(2pi) + logdet(Sigma_k)
+ es_k(t)^T Sigma_k^{-1} es_k(t)) with es = causal_conv(x, W, b).  Since
Sigma^{-1} = L^{-T} L^{-1} and es is affine in x, fold L^{-1} into the conv:
W2 = L^{-1} W, b2 = L^{-1} b, then mahalanobis = sum_c conv(x; W2, b2)^2.

fp8 DoubleRow version: conv matmuls run in fp8e4 (e4m3) with
perf_mode=DoubleRow, which packs 2 contraction rows per PE cell (virtual
128x256 array).  Contraction of 577 rows (9 taps x 64 cin + bias) per output
chunk is packed as 2 full DR steps of 256 virtual rows (taps 0-7; partition
p = (cin, g), pair slot i covers tap 4s+2g+i) plus one DR leftover step
(tap 8 as channel-pairs on 32 partitions + a ones/bias row).  PSUM chunk is
[128 t, 512 (8 states x 64 ch)] x 2 halves.  ACT squares PSUM -> bf16 SBUF
with the free scale folding in 1/(Sx*Sw*sqrt(2)); DVE does the per-state
segmented reduce with negate ([128,8,64] -> -[128,8]); a small PE transpose
flips [128 t, 16 k] -> [16 k, 128 t] batched 4 chunks per PSUM tile; ACT
adds the per-state constant; DMA out.
"""

import math
import os

import numpy as np
import ml_dtypes

import concourse.bass as bass
import concourse.bacc as bacc_mod
import concourse.mybir as mybir
import concourse.tile as tile
from concourse.bass_utils import run_bass_kernel_spmd
from concourse.tile_rust import add_dep_helper

K = 16          # states
C = 64          # channels
T = 65536       # time
AR = 8          # ar order (kernel size AR+1)
NCORES = 8
TLOC = T // NCORES          # 8192 outputs per core
TC = 128                    # outputs per chunk (matmul M)
WAVE = 16                   # chunks per wave (input tile granularity)
WCOLS = TC * WAVE           # 2048 outputs per wave
NW = TLOC // WCOLS          # waves per core
NH = 2                      # psum halves (states 0-7, 8-15)
NS = 2                      # full DoubleRow contraction steps (taps 0-7)
MTGRP = 4                   # chunks batched per [16, 512] transpose psum

# DoubleRow LDWEIGHTS requires the pair-region byte stride % 16 == 0
XWW = WCOLS + 16            # xq wave-tile region width (max col 2051 used)
XQW = (NW - 1) * WCOLS + XWW  # xq dram region width

FP8 = mybir.dt.float8e4
SQ_DT = mybir.dt.bfloat16   # squares dtype
DR = mybir.MatmulPerfMode.DoubleRow

SX = 16.0                   # x scale into fp8
SW = 64.0                   # weight scale into fp8
ACT_SCALE = 1.0 / (SX * SW * math.sqrt(2.0))

_FP8_NP = ml_dtypes.float8_e4m3
FP8_MAX = 240.0

_CACHE: dict = {}


def _build_program():
    nc = bacc_mod.Bacc()
    f32 = mybir.dt.float32

    # xq rows p=(c,g): 2 regions i: x[c, t0-8 + a + 2g + i] * SX (taps 0-7)
    xq = nc.declare_dram_parameter("xq", [128, 2, XQW], FP8, isOutput=False)
    # xe rows p<32: 2 regions i: x[2p+i, t0 + a] * SX (tap 8);
    # row 32: (ones, zeros); rows 33-63: zeros; rows 64-127: replica of 0-63
    # (leftover matmuls of adjacent chunks run concurrently in PE row-groups
    # {0,1} and {2,3} via tile_position, so both operand copies are needed)
    xe = nc.declare_dram_parameter("xe", [128, 2, TLOC], FP8, isOutput=False)
    # moving operands store DoubleRow pairs interleaved (contiguous byte
    # pairs stream at full rate; split regions force 2 fetches/cycle)
    # wts[p, s, n, i] = SW * W2[k(n), d(n), c(p), 4s + 2g(p) + i]
    wts = nc.declare_dram_parameter("wts", [128, NS, 1024, 2], FP8, isOutput=False)
    # web[p, n, i] = SW * W2[k, d, 2p+i, 8] (p<32); row 32 i=0: SX*SW*b2;
    # rows 64-127: replica of 0-63
    web = nc.declare_dram_parameter("web", [128, 1024, 2], FP8, isOutput=False)
    ident = nc.declare_dram_parameter("ident", [128, 128], mybir.dt.float32r, isOutput=False)
    biasc = nc.declare_dram_parameter("biasc", [K, 1], f32, isOutput=False)
    out = nc.declare_dram_parameter("out", [K, TLOC], f32, isOutput=True)

    with tile.TileContext(nc) as tc:
        with (
            tc.tile_pool(name="singles", bufs=1) as singles,
            # one slot per wave: input DMAs never wait (no slot WAR/WAW)
            tc.tile_pool(name="xpool", bufs=NW) as xpool,
            tc.tile_pool(name="sqpool", bufs=12) as sqpool,
            tc.tile_pool(name="mpool", bufs=6) as mpool,
            tc.tile_pool(name="conv_ps", bufs=6, space="PSUM") as conv_ps,
            tc.tile_pool(name="mt_ps", bufs=1, space="PSUM") as mt_ps,
            tc.tile_pool(name="obs_ps", bufs=1, space="PSUM") as obs_ps,
        ):
            # Matmuls must never be the first PE instruction to observe more
            # than one producer semaphore (1-wait ISA slots; bacc's event-sem
            # legalization costs sequencer time).  pe_observe() emits a tiny
            # 2x2 "reader" matmul whose operands come from a single
            # producer's tile; ordering edges pin readers ahead of the next
            # real matmul.
            scratch = obs_ps.tile([2, 128], f32)
            scratch2 = singles.tile([2, 128], SQ_DT)
            nc.vector.memset(scratch2, 0.0)
            pending = []
            obs_after = [None]

            def pe_observe(col):
                i = nc.tensor.matmul(
                    scratch[0:2, 0:2], col, col, start=True, stop=True
                )
                if obs_after[0] is not None:
                    # not earlier than late in the previous wave, or the PE
                    # FIFO head-of-line blocks on a DMA that hasn't landed
                    add_dep_helper(i.ins, obs_after[0].ins, sync=False)
                pending.append(i)

            def _flush(i):
                while pending:
                    add_dep_helper(i.ins, pending.pop().ins, sync=False)
                return i

            def pe_matmul(*args, **kw):
                return _flush(nc.tensor.matmul(*args, **kw))

            # dep-free warmup matmuls: keep the PE busy through the initial
            # input DMAs so HAM un-throttles before real work
            for _ in range(35):
                nc.tensor.matmul(
                    scratch[0:2, 0:128],
                    scratch2[0:2, 0:2],
                    scratch2[0:2, 0:128],
                    start=True,
                    stop=True,
                )

            # DMA issue plan: sync HWDGE ring carries the critical path
            # (first xq piece, weights, rest of xq); prefetchables
            # (identity, bias, xe/web, waves 1+) go on the scalar ring.
            w_sb = singles.tile([128, NS, 1024, 2], FP8)
            web_sb = singles.tile([128, 1024, 2], FP8)
            ident_sb = singles.tile([128, 128], mybir.dt.float32r)
            bias_sb = singles.tile([K, 1], f32)
            out_sb = singles.tile([K, TLOC], f32)
            xqs, xes = [], []
            sc_dmas = []
            sc_dmas.append(nc.scalar.dma_start(out=ident_sb, in_=ident[:, :]))
            sc_dmas.append(nc.scalar.dma_start(out=bias_sb, in_=biasc[:, :]))
            for w in range(NW):
                base = w * WCOLS
                xq_w = xpool.tile([128, 2, XWW], FP8, name="xq_w")
                xe_w = xpool.tile([128, 2, WCOLS], FP8, name="xe_w")
                if w == 0:
                    # first piece covers chunks 0-1 (cols < 260)
                    nc.sync.dma_start(out=xq_w[:, :, 0:264], in_=xq[:, :, 0:264])
                    nc.sync.dma_start(out=w_sb, in_=wts[:, :, :, :])
                    w_mv = w_sb.rearrange("p s n i -> p s i n")
                    web_mv = web_sb.rearrange("p n i -> p i n")
                    nc.sync.dma_start(
                        out=xq_w[:, :, 264:XWW], in_=xq[:, :, 264:XWW]
                    )
                    sc_dmas.append(
                        nc.scalar.dma_start(
                            out=xe_w, in_=xe[:, :, base : base + WCOLS]
                        )
                    )
                    sc_dmas.append(nc.scalar.dma_start(out=web_sb, in_=web[:, :, :]))
                elif w == 1:
                    sc_dmas.append(
                        nc.scalar.dma_start(
                            out=xq_w, in_=xq[:, :, base : base + XWW]
                        )
                    )
                    sc_dmas.append(
                        nc.scalar.dma_start(
                            out=xe_w, in_=xe[:, :, base : base + WCOLS]
                        )
                    )
                xqs.append(xq_w)
                xes.append(xe_w)

            def load_wave_inputs(w):
                # waves 2-3 load lazily (two waves ahead) so the prefetch
                # doesn't flood the DMA fabric while wave 0 computes
                base = w * WCOLS
                nc.scalar.dma_start(out=xqs[w], in_=xq[:, :, base : base + XWW])
                nc.scalar.dma_start(out=xes[w], in_=xe[:, :, base : base + WCOLS])

            # DVE observer for the bias DMA (TS struct fits one wait)
            dve_scratch = singles.tile([K, 1], f32)
            nc.vector.tensor_copy(dve_scratch, bias_sb)

            first_sq = [True]
            m_sbs = []

            def chunk_tail(cglob, psh):
                m_sb = mpool.tile([128, K], mybir.dt.float32r, name="m_sb")
                for h in range(NH):
                    sq = sqpool.tile([128, 512], SQ_DT, name="sq", tag="sq")
                    sq_i = nc.scalar.activation(
                        sq,
                        psh[h],
                        mybir.ActivationFunctionType.Square,
                        scale=ACT_SCALE,
                    )
                    if first_sq[0]:
                        # the Act sequencer must issue every prefetch DMA
                        # before its first square, else a square that
                        # transitively gates one of those DMAs deadlocks
                        while sc_dmas:
                            add_dep_helper(sq_i.ins, sc_dmas.pop().ins, sync=False)
                        first_sq[0] = False
                    with nc.allow_low_precision(
                        reason="float32r shares float32 bits; r-mode only "
                        "affects the PE multiply path"
                    ):
                        nc.vector.tensor_reduce(
                            out=m_sb[:, 8 * h : 8 * h + 8],
                            in_=sq.rearrange("p (g c) -> p g c", g=8),
                            axis=mybir.AxisListType.X,
                            op=mybir.AluOpType.add,
                            negate=True,
                        )
                m_sbs.append(m_sb)
                if cglob % MTGRP == MTGRP - 1:
                    # batch the group's transposes back-to-back so the ident
                    # stationary is loaded once per group, not per chunk
                    mt = mt_ps.tile([K, MTGRP * TC], mybir.dt.float32r, name="mt")
                    for g in range(MTGRP):
                        _flush(
                            nc.tensor.transpose(
                                mt[:, g * TC : (g + 1) * TC], m_sbs[g], ident_sb
                            )
                        )
                    m_sbs.clear()
                    gbase = (cglob - MTGRP + 1) * TC
                    # out = -m/2 + (-0.5*(Dlog2pi + logdet))  on ACT
                    nc.scalar.activation(
                        out_sb[:, gbase : gbase + MTGRP * TC],
                        mt[0:K, :],
                        mybir.ActivationFunctionType.Identity,
                        bias=bias_sb,
                        scale=1.0,
                    )

            for w in range(NW):
                base = w * WCOLS
                xq_w = xqs[w]
                xe_w = xes[w]
                if w + 2 < NW:
                    load_wave_inputs(w + 2)
                for pc in range(WAVE // 2):
                    cpair = (2 * pc, 2 * pc + 1)
                    psh = {}
                    for cc in cpair:
                        off = cc * TC
                        psh[cc] = [
                            conv_ps.tile(
                                [128, 512], mybir.dt.float32, name=f"ps{h}", tag="ps"
                            )
                            for h in range(NH)
                        ]
                        if cc == 0:
                            if w == 0:
                                pe_observe(ident_sb[:, 0:2])
                            pe_observe(xq_w[0:2, 0, 0:2])
                        for s in range(NS):
                            lhsT = xq_w[:, :, off + 4 * s : off + 4 * s + TC]
                            for h in range(NH):
                                pe_matmul(
                                    psh[cc][h],
                                    lhsT,
                                    w_mv[:, s, :, 512 * h : 512 * h + 512],
                                    start=(s == 0),
                                    stop=False,
                                    perf_mode=DR,
                                )
                    if pc == 0:
                        # lazily: s0-s1 must not stall on the xe/web loads
                        pe_observe(xe_w[0:2, 0, 0:2])
                        pe_observe(web_sb[0:2, 0, 0:2])
                    # leftovers (tap 8 + bias): K=64 row-group tiles; chunk
                    # pair runs concurrently in PE row-groups {0,1} / {2,3}
                    for h in range(NH):
                        for ci, cc in enumerate(cpair):
                            off = cc * TC
                            bp = 64 * ci
                            mm_i = pe_matmul(
                                psh[cc][h],
                                xe_w[bp : bp + 64, :, off : off + TC],
                                web_mv[bp : bp + 64, :, 512 * h : 512 * h + 512],
                                start=False,
                                stop=True,
                                perf_mode=DR,
                                tile_position=(bp, 0),
                            )
                    if pc == WAVE // 2 - 1:
                        obs_after[0] = mm_i
                    for cc in cpair:
                        chunk_tail(w * WAVE + cc, psh[cc])
                if w < NW - 1:
                    nc.sync.dma_start(
                        out=out[:, base : base + WCOLS],
                        in_=out_sb[:, base : base + WCOLS],
                    )
                else:
                    # last wave: quarter DMAs so the final store is tiny
                    for q in range(4):
                        uq = base + q * (WCOLS // 4)
                        nc.sync.dma_start(
                            out=out[:, uq : uq + WCOLS // 4],
                            in_=out_sb[:, uq : uq + WCOLS // 4],
                        )
    nc.compile()
    return nc


def _prep_host(W, b, Sigma):
    """Fold L^{-1} into conv weights; pack fp8 DoubleRow tiles, constants."""
    W64 = W.astype(np.float64)
    b64 = b.astype(np.float64)
    S64 = Sigma.astype(np.float64)
    L = np.linalg.cholesky(S64)
    Li = np.linalg.inv(L)                       # [K, C, C] lower-triangular inv
    logdet = 2.0 * np.sum(np.log(np.diagonal(L, axis1=1, axis2=2)), axis=1)
    W2 = np.einsum("kdc,kcij->kdij", Li, W64)   # [K, C(d), C(ci), 9]
    b2 = np.einsum("kdc,kc->kd", Li, b64)       # [K, C]

    def q8(v):
        return np.clip(v, -FP8_MAX, FP8_MAX).astype(_FP8_NP)

    # weight column layout: n = 512*(k//8) + 64*(k%8) + d
    W2n = np.transpose(W2, (0, 1, 3, 2)).reshape(K * C, 9, C)  # [(k,d), j, c]
    W2n = W2n.reshape(2, 512, 9, C)                            # [h, n', j, c]

    # wts[p, i, s, n] = SW * W2[k(n), d(n), c(p), 4s + 2g(p) + i]
    wts_np = np.zeros((128, 2, NS, 1024), np.float64)
    for g in range(2):
        for i in range(2):
            for s in range(NS):
                j = 4 * s + 2 * g + i
                # [c, (h, n')] for tap j
                wj = np.transpose(W2n[:, :, j, :], (2, 0, 1)).reshape(C, 1024)
                wts_np[64 * g : 64 * g + 64, i, s, :] = SW * wj
    # web[p, i, n] = SW * W2[k, d, 2p+i, 8] (p<32); row 32 i=0: SX*SW*b2
    web_np = np.zeros((128, 2, 1024), np.float64)
    w8 = np.transpose(W2n[:, :, 8, :], (2, 0, 1)).reshape(C, 1024)  # [c, n]
    web_np[0:32, 0, :] = SW * w8[0::2, :]
    web_np[0:32, 1, :] = SW * w8[1::2, :]
    web_np[32, 0, :] = SX * SW * b2.reshape(2, 8, 64).reshape(1024)
    web_np[64:128] = web_np[0:64]

    const = C * np.log(2.0 * np.pi) + logdet
    bias_np = (-0.5 * const).astype(np.float32).reshape(K, 1)
    # interleave DoubleRow pairs along the innermost byte
    wts_il = np.ascontiguousarray(np.transpose(wts_np, (0, 2, 3, 1)))
    web_il = np.ascontiguousarray(np.transpose(web_np, (0, 2, 1)))
    return q8(wts_il), q8(web_il), bias_np


def _run(x, W, b, Sigma, trace=False):
    x = np.asarray(x, np.float32)
    W = np.asarray(W, np.float32)
    b = np.asarray(b, np.float32)
    Sigma = np.asarray(Sigma, np.float32)
    if "nc" not in _CACHE:
        _CACHE["nc"] = _build_program()
    nc = _CACHE["nc"]
    wts_np, web_np, bias_np = _prep_host(W, b, Sigma)

    # causal left pad (AR) plus right pad so shifted copies stay in bounds
    xp = np.pad(x[0].astype(np.float64), ((0, 0), (AR, 24)))       # [C, T+32]
    xp8 = np.clip(SX * xp, -FP8_MAX, FP8_MAX).astype(_FP8_NP)
    ident_np = np.eye(128, dtype=np.float32)
    in_maps = []
    for ci in range(NCORES):
        t0 = ci * TLOC
        # xq[p, i, a] = xp8[c, t0 + a + 2g + i]
        xq_np = np.zeros((128, 2, XQW), _FP8_NP)
        for g in range(2):
            for i in range(2):
                sh = 2 * g + i
                xq_np[64 * g : 64 * g + 64, i, :] = xp8[:, t0 + sh : t0 + sh + XQW]
        # xe[p, i, a] = xp8[2p+i, t0 + 8 + a] (p<32); row 32 = (1, 0);
        # rows 64-127 replicate 0-63 for the second PE row-group
        xe_np = np.zeros((128, 2, TLOC), _FP8_NP)
        xe_np[0:32, 0, :] = xp8[0::2, t0 + 8 : t0 + 8 + TLOC]
        xe_np[0:32, 1, :] = xp8[1::2, t0 + 8 : t0 + 8 + TLOC]
        xe_np[32, 0, :] = _FP8_NP(1.0)
        xe_np[64:128] = xe_np[0:64]
        in_maps.append(
            {
                "xq": xq_np,
                "xe": xe_np,
                "wts": wts_np,
                "web": web_np,
                "ident": ident_np,
                "biasc": bias_np,
            }
        )
    res = run_bass_kernel_spmd(
        nc, in_maps, core_ids=list(range(NCORES)), trace=trace
    )
    outs = [res.results[i]["out"] for i in range(NCORES)]
    full = np.concatenate(outs, axis=1)[None]   # [1, K, T]
    return full.astype(np.float32), res


def kernel(x, W, b, Sigma):
    out, _ = _run(x, W, b, Sigma, trace=bool(int(os.environ.get("BASS_TRACE", "0"))))
    return out


# revision 24
# speedup vs baseline: 1.2812x; 1.1584x over previous
"""Trainium2 Bass kernel for nn_Autoregression (16-state AR whitening log-prob).

Math: reference computes log_prob[b,k,t] = -0.5*(C*log(2pi) + logdet(Sigma_k)
+ es_k(t)^T Sigma_k^{-1} es_k(t)) with es = causal_conv(x, W, b).  Since
Sigma^{-1} = L^{-T} L^{-1} and es is affine in x, fold L^{-1} into the conv:
W2 = L^{-1} W, b2 = L^{-1} b, then mahalanobis = sum_c conv(x; W2, b2)^2.

fp8 DoubleRow version: conv matmuls run in fp8e4 (e4m3) with
perf_mode=DoubleRow, which packs 2 contraction rows per PE cell (virtual
128x256 array).  Contraction of 577 rows (9 taps x 64 cin + bias) per output
chunk is packed as 2 full DR steps of 256 virtual rows (taps 0-7; partition
p = (cin, g), pair slot i covers tap 4s+2g+i) plus one DR leftover step
(tap 8 as channel-pairs on 32 partitions + a ones/bias row).  PSUM chunk is
[128 t, 512 (8 states x 64 ch)] x 2 halves.  ACT squares PSUM -> bf16 SBUF
with the free scale folding in 1/(Sx*Sw*sqrt(2)); DVE does the per-state
segmented reduce with negate ([128,8,64] -> -[128,8]); a small PE transpose
flips [128 t, 16 k] -> [16 k, 128 t] batched 4 chunks per PSUM tile; ACT
adds the per-state constant; DMA out.
"""

import math
import os

import numpy as np
import ml_dtypes

import concourse.bass as bass
import concourse.bacc as bacc_mod
import concourse.mybir as mybir
import concourse.tile as tile
from concourse.bass_utils import run_bass_kernel_spmd
from concourse.tile_rust import add_dep_helper

K = 16          # states
C = 64          # channels
T = 65536       # time
AR = 8          # ar order (kernel size AR+1)
NCORES = 8
TLOC = T // NCORES          # 8192 outputs per core
TC = 128                    # outputs per chunk (matmul M)
WAVE = 16                   # chunks per wave (input tile granularity)
WCOLS = TC * WAVE           # 2048 outputs per wave
NW = TLOC // WCOLS          # waves per core
NH = 2                      # psum halves (states 0-7, 8-15)
NS = 2                      # full DoubleRow contraction steps (taps 0-7)
MTGRP = 4                   # chunks batched per [16, 512] transpose psum

# DoubleRow LDWEIGHTS requires the pair-region byte stride % 16 == 0
XWW = WCOLS + 16            # xq wave-tile region width (max col 2051 used)
XQW = (NW - 1) * WCOLS + XWW  # xq dram region width

FP8 = mybir.dt.float8e4
SQ_DT = mybir.dt.bfloat16   # squares dtype
DR = mybir.MatmulPerfMode.DoubleRow

SX = 16.0                   # x scale into fp8
SW = 64.0                   # weight scale into fp8
ACT_SCALE = 1.0 / (SX * SW * math.sqrt(2.0))

_FP8_NP = ml_dtypes.float8_e4m3
FP8_MAX = 240.0

_CACHE: dict = {}


def _build_program():
    nc = bacc_mod.Bacc()
    f32 = mybir.dt.float32

    # xq rows p=(c,g): 2 regions i: x[c, t0-8 + a + 2g + i] * SX (taps 0-7)
    xq = nc.declare_dram_parameter("xq", [128, 2, XQW], FP8, isOutput=False)
    # xe rows p<32: 2 regions i: x[2p+i, t0 + a] * SX (tap 8);
    # row 32: (ones, zeros); rows 33-63: zeros; rows 64-127: replica of 0-63
    # (leftover matmuls of adjacent chunks run concurrently in PE row-groups
    # {0,1} and {2,3} via tile_position, so both operand copies are needed)
    xe = nc.declare_dram_parameter("xe", [128, 2, TLOC], FP8, isOutput=False)
    # moving operands store DoubleRow pairs interleaved (contiguous byte
    # pairs stream at full rate; split regions force 2 fetches/cycle)
    # wts[p, s, n, i] = SW * W2[k(n), d(n), c(p), 4s + 2g(p) + i]
    wts = nc.declare_dram_parameter("wts", [128, NS, 1024, 2], FP8, isOutput=False)
    # web[p, n, i] = SW * W2[k, d, 2p+i, 8] (p<32); row 32 i=0: SX*SW*b2;
    # rows 64-127: replica of 0-63
    web = nc.declare_dram_parameter("web", [128, 1024, 2], FP8, isOutput=False)
    ident = nc.declare_dram_parameter("ident", [128, 128], mybir.dt.float32r, isOutput=False)
    biasc = nc.declare_dram_parameter("biasc", [K, 1], f32, isOutput=False)
    out = nc.declare_dram_parameter("out", [K, TLOC], f32, isOutput=True)

    with tile.TileContext(nc) as tc:
        with (
            tc.tile_pool(name="singles", bufs=1) as singles,
            # one slot per wave: input DMAs never wait (no slot WAR/WAW)
            tc.tile_pool(name="xpool", bufs=NW) as xpool,
            tc.tile_pool(name="sqpool", bufs=12) as sqpool,
            tc.tile_pool(name="mpool", bufs=10) as mpool,
            tc.tile_pool(name="conv_ps", bufs=6, space="PSUM") as conv_ps,
            tc.tile_pool(name="mt_ps", bufs=1, space="PSUM") as mt_ps,
            tc.tile_pool(name="obs_ps", bufs=1, space="PSUM") as obs_ps,
        ):
            # Matmuls must never be the first PE instruction to observe more
            # than one producer semaphore (1-wait ISA slots; bacc's event-sem
            # legalization costs sequencer time).  pe_observe() emits a tiny
            # 2x2 "reader" matmul whose operands come from a single
            # producer's tile; ordering edges pin readers ahead of the next
            # real matmul.
            scratch = obs_ps.tile([2, 128], f32)
            scratch2 = singles.tile([2, 128], SQ_DT)
            nc.vector.memset(scratch2, 0.0)
            pending = []
            obs_after = [None]

            def pe_observe(col):
                i = nc.tensor.matmul(
                    scratch[0:2, 0:2], col, col, start=True, stop=True
                )
                if obs_after[0] is not None:
                    # not earlier than late in the previous wave, or the PE
                    # FIFO head-of-line blocks on a DMA that hasn't landed
                    add_dep_helper(i.ins, obs_after[0].ins, sync=False)
                pending.append(i)

            def _flush(i):
                while pending:
                    add_dep_helper(i.ins, pending.pop().ins, sync=False)
                return i

            # the tile scheduler reorders PE instructions by priority, which
            # scatters stationary-operand switches; chain every real PE op in
            # emission order so LDWEIGHTS double-buffering can hide switches
            chain = [None]

            def _chain(i):
                if chain[0] is not None:
                    add_dep_helper(i.ins, chain[0].ins, sync=False)
                chain[0] = i
                return i

            def pe_matmul(*args, **kw):
                return _chain(_flush(nc.tensor.matmul(*args, **kw)))

            # dep-free warmup matmuls: keep the PE busy through the initial
            # input DMAs so HAM un-throttles before real work
            for _ in range(35):
                nc.tensor.matmul(
                    scratch[0:2, 0:128],
                    scratch2[0:2, 0:2],
                    scratch2[0:2, 0:128],
                    start=True,
                    stop=True,
                )

            # DMA issue plan: sync HWDGE ring carries the critical path
            # (first xq piece, weights, rest of xq); prefetchables
            # (identity, bias, xe/web, waves 1+) go on the scalar ring.
            w_sb = singles.tile([128, NS, 1024, 2], FP8)
            web_sb = singles.tile([128, 1024, 2], FP8)
            ident_sb = singles.tile([128, 128], mybir.dt.float32r)
            bias_sb = singles.tile([K, 1], f32)
            out_sb = singles.tile([K, TLOC], f32)
            xqs, xes = [], []
            sc_dmas = []
            sc_dmas.append(nc.scalar.dma_start(out=ident_sb, in_=ident[:, :]))
            sc_dmas.append(nc.scalar.dma_start(out=bias_sb, in_=biasc[:, :]))
            for w in range(NW):
                base = w * WCOLS
                xq_w = xpool.tile([128, 2, XWW], FP8, name="xq_w")
                xe_w = xpool.tile([128, 2, WCOLS], FP8, name="xe_w")
                if w == 0:
                    # first piece covers chunks 0-1 (cols < 260)
                    nc.sync.dma_start(out=xq_w[:, :, 0:264], in_=xq[:, :, 0:264])
                    nc.sync.dma_start(out=w_sb, in_=wts[:, :, :, :])
                    w_mv = w_sb.rearrange("p s n i -> p s i n")
                    web_mv = web_sb.rearrange("p n i -> p i n")
                    nc.sync.dma_start(
                        out=xq_w[:, :, 264:XWW], in_=xq[:, :, 264:XWW]
                    )
                    sc_dmas.append(
                        nc.scalar.dma_start(
                            out=xe_w, in_=xe[:, :, base : base + WCOLS]
                        )
                    )
                    sc_dmas.append(nc.scalar.dma_start(out=web_sb, in_=web[:, :, :]))
                elif w == 1:
                    sc_dmas.append(
                        nc.scalar.dma_start(
                            out=xq_w, in_=xq[:, :, base : base + XWW]
                        )
                    )
                    sc_dmas.append(
                        nc.scalar.dma_start(
                            out=xe_w, in_=xe[:, :, base : base + WCOLS]
                        )
                    )
                xqs.append(xq_w)
                xes.append(xe_w)

            def load_wave_inputs(w):
                # waves 2-3 load lazily (two waves ahead) so the prefetch
                # doesn't flood the DMA fabric while wave 0 computes
                base = w * WCOLS
                nc.scalar.dma_start(out=xqs[w], in_=xq[:, :, base : base + XWW])
                nc.scalar.dma_start(out=xes[w], in_=xe[:, :, base : base + WCOLS])

            # DVE observer for the bias DMA (TS struct fits one wait)
            dve_scratch = singles.tile([K, 1], f32)
            nc.vector.tensor_copy(dve_scratch, bias_sb)

            first_sq = [True]
            m_sbs = []
            pending_tails = []

            def chunk_tail(cglob, psh):
                m_sb = mpool.tile([128, K], mybir.dt.float32r, name="m_sb")
                for h in range(NH):
                    sq = sqpool.tile([128, 512], SQ_DT, name="sq", tag="sq")
                    sq_i = nc.scalar.activation(
                        sq,
                        psh[h],
                        mybir.ActivationFunctionType.Square,
                        scale=ACT_SCALE,
                    )
                    if first_sq[0]:
                        # the Act sequencer must issue every prefetch DMA
                        # before its first square, else a square that
                        # transitively gates one of those DMAs deadlocks
                        while sc_dmas:
                            add_dep_helper(sq_i.ins, sc_dmas.pop().ins, sync=False)
                        first_sq[0] = False
                    with nc.allow_low_precision(
                        reason="float32r shares float32 bits; r-mode only "
                        "affects the PE multiply path"
                    ):
                        nc.vector.tensor_reduce(
                            out=m_sb[:, 8 * h : 8 * h + 8],
                            in_=sq.rearrange("p (g c) -> p g c", g=8),
                            axis=mybir.AxisListType.X,
                            op=mybir.AluOpType.add,
                            negate=True,
                        )
                m_sbs.append(m_sb)
                if cglob % MTGRP == MTGRP - 1:
                    pending_tails.append((cglob - MTGRP + 1, list(m_sbs)))
                    m_sbs.clear()

            def emit_tail():
                # batched transposes (ident stationary loaded once per group),
                # chained into the PE stream a pair late so the DVE reduces
                # they read have landed and the PE never stalls on them
                gbase_c, msbs = pending_tails.pop(0)
                mt = mt_ps.tile([K, MTGRP * TC], mybir.dt.float32r, name="mt")
                for g in range(MTGRP):
                    _chain(
                        _flush(
                            nc.tensor.transpose(
                                mt[:, g * TC : (g + 1) * TC], msbs[g], ident_sb
                            )
                        )
                    )
                gbase = gbase_c * TC
                # out = -m/2 + (-0.5*(Dlog2pi + logdet))  on ACT
                nc.scalar.activation(
                    out_sb[:, gbase : gbase + MTGRP * TC],
                    mt[0:K, :],
                    mybir.ActivationFunctionType.Identity,
                    bias=bias_sb,
                    scale=1.0,
                )
                # the store must be emitted after the ACT write or the tile
                # framework orders it before (WAR) and ships stale columns
                nc.sync.dma_start(
                    out=out[:, gbase : gbase + MTGRP * TC],
                    in_=out_sb[:, gbase : gbase + MTGRP * TC],
                )

            for w in range(NW):
                base = w * WCOLS
                xq_w = xqs[w]
                xe_w = xes[w]
                if w + 2 < NW:
                    load_wave_inputs(w + 2)
                for pc in range(WAVE // 2):
                    cpair = (2 * pc, 2 * pc + 1)
                    psh = {}
                    for cc in cpair:
                        off = cc * TC
                        psh[cc] = [
                            conv_ps.tile(
                                [128, 512], mybir.dt.float32, name=f"ps{h}", tag="ps"
                            )
                            for h in range(NH)
                        ]
                        if cc == 0:
                            if w == 0:
                                pe_observe(ident_sb[:, 0:2])
                            pe_observe(xq_w[0:2, 0, 0:2])
                        for s in range(NS):
                            lhsT = xq_w[:, :, off + 4 * s : off + 4 * s + TC]
                            for h in range(NH):
                                pe_matmul(
                                    psh[cc][h],
                                    lhsT,
                                    w_mv[:, s, :, 512 * h : 512 * h + 512],
                                    start=(s == 0),
                                    stop=False,
                                    perf_mode=DR,
                                )
                    if pc == 0:
                        # lazily: s0-s1 must not stall on the xe/web loads
                        pe_observe(xe_w[0:2, 0, 0:2])
                        pe_observe(web_sb[0:2, 0, 0:2])
                    # leftovers (tap 8 + bias): K=64 row-group tiles; chunk
                    # pair runs concurrently in PE row-groups {0,1} / {2,3}
                    for h in range(NH):
                        for ci, cc in enumerate(cpair):
                            off = cc * TC
                            bp = 64 * ci
                            mm_i = pe_matmul(
                                psh[cc][h],
                                xe_w[bp : bp + 64, :, off : off + TC],
                                web_mv[bp : bp + 64, :, 512 * h : 512 * h + 512],
                                start=False,
                                stop=True,
                                perf_mode=DR,
                                tile_position=(bp, 0),
                            )
                    if pc == WAVE // 2 - 1:
                        obs_after[0] = mm_i
                    for cc in cpair:
                        chunk_tail(w * WAVE + cc, psh[cc])
                    # emit lagged transpose batches; drain fully at the end
                    last = w == NW - 1 and pc == WAVE // 2 - 1
                    while pending_tails and (len(pending_tails) > 1 or last):
                        emit_tail()
    nc.compile()
    return nc


def _prep_host(W, b, Sigma):
    """Fold L^{-1} into conv weights; pack fp8 DoubleRow tiles, constants."""
    W64 = W.astype(np.float64)
    b64 = b.astype(np.float64)
    S64 = Sigma.astype(np.float64)
    L = np.linalg.cholesky(S64)
    Li = np.linalg.inv(L)                       # [K, C, C] lower-triangular inv
    logdet = 2.0 * np.sum(np.log(np.diagonal(L, axis1=1, axis2=2)), axis=1)
    W2 = np.einsum("kdc,kcij->kdij", Li, W64)   # [K, C(d), C(ci), 9]
    b2 = np.einsum("kdc,kc->kd", Li, b64)       # [K, C]

    def q8(v):
        return np.clip(v, -FP8_MAX, FP8_MAX).astype(_FP8_NP)

    # weight column layout: n = 512*(k//8) + 64*(k%8) + d
    W2n = np.transpose(W2, (0, 1, 3, 2)).reshape(K * C, 9, C)  # [(k,d), j, c]
    W2n = W2n.reshape(2, 512, 9, C)                            # [h, n', j, c]

    # wts[p, i, s, n] = SW * W2[k(n), d(n), c(p), 4s + 2g(p) + i]
    wts_np = np.zeros((128, 2, NS, 1024), np.float64)
    for g in range(2):
        for i in range(2):
            for s in range(NS):
                j = 4 * s + 2 * g + i
                # [c, (h, n')] for tap j
                wj = np.transpose(W2n[:, :, j, :], (2, 0, 1)).reshape(C, 1024)
                wts_np[64 * g : 64 * g + 64, i, s, :] = SW * wj
    # web[p, i, n] = SW * W2[k, d, 2p+i, 8] (p<32); row 32 i=0: SX*SW*b2
    web_np = np.zeros((128, 2, 1024), np.float64)
    w8 = np.transpose(W2n[:, :, 8, :], (2, 0, 1)).reshape(C, 1024)  # [c, n]
    web_np[0:32, 0, :] = SW * w8[0::2, :]
    web_np[0:32, 1, :] = SW * w8[1::2, :]
    web_np[32, 0, :] = SX * SW * b2.reshape(2, 8, 64).reshape(1024)
    web_np[64:128] = web_np[0:64]

    const = C * np.log(2.0 * np.pi) + logdet
    bias_np = (-0.5 * const).astype(np.float32).reshape(K, 1)
    # interleave DoubleRow pairs along the innermost byte
    wts_il = np.ascontiguousarray(np.transpose(wts_np, (0, 2, 3, 1)))
    web_il = np.ascontiguousarray(np.transpose(web_np, (0, 2, 1)))
    return q8(wts_il), q8(web_il), bias_np


def _run(x, W, b, Sigma, trace=False):
    x = np.asarray(x, np.float32)
    W = np.asarray(W, np.float32)
    b = np.asarray(b, np.float32)
    Sigma = np.asarray(Sigma, np.float32)
    if "nc" not in _CACHE:
        _CACHE["nc"] = _build_program()
    nc = _CACHE["nc"]
    wts_np, web_np, bias_np = _prep_host(W, b, Sigma)

    # causal left pad (AR) plus right pad so shifted copies stay in bounds
    xp = np.pad(x[0].astype(np.float64), ((0, 0), (AR, 24)))       # [C, T+32]
    xp8 = np.clip(SX * xp, -FP8_MAX, FP8_MAX).astype(_FP8_NP)
    ident_np = np.eye(128, dtype=np.float32)
    in_maps = []
    for ci in range(NCORES):
        t0 = ci * TLOC
        # xq[p, i, a] = xp8[c, t0 + a + 2g + i]
        xq_np = np.zeros((128, 2, XQW), _FP8_NP)
        for g in range(2):
            for i in range(2):
                sh = 2 * g + i
                xq_np[64 * g : 64 * g + 64, i, :] = xp8[:, t0 + sh : t0 + sh + XQW]
        # xe[p, i, a] = xp8[2p+i, t0 + 8 + a] (p<32); row 32 = (1, 0);
        # rows 64-127 replicate 0-63 for the second PE row-group
        xe_np = np.zeros((128, 2, TLOC), _FP8_NP)
        xe_np[0:32, 0, :] = xp8[0::2, t0 + 8 : t0 + 8 + TLOC]
        xe_np[0:32, 1, :] = xp8[1::2, t0 + 8 : t0 + 8 + TLOC]
        xe_np[32, 0, :] = _FP8_NP(1.0)
        xe_np[64:128] = xe_np[0:64]
        in_maps.append(
            {
                "xq": xq_np,
                "xe": xe_np,
                "wts": wts_np,
                "web": web_np,
                "ident": ident_np,
                "biasc": bias_np,
            }
        )
    res = run_bass_kernel_spmd(
        nc, in_maps, core_ids=list(range(NCORES)), trace=trace
    )
    outs = [res.results[i]["out"] for i in range(NCORES)]
    full = np.concatenate(outs, axis=1)[None]   # [1, K, T]
    return full.astype(np.float32), res


def kernel(x, W, b, Sigma):
    out, _ = _run(x, W, b, Sigma, trace=bool(int(os.environ.get("BASS_TRACE", "0"))))
    return out
